# revision 1
# baseline (speedup 1.0000x reference)
"""AttnBlock (GroupNorm + 4-head d=128 self-attention + residual).

Full input x: [8, 512, 2048] fp32. Data-parallel over batch: core b computes
batch b entirely on-chip (no collectives).

Per-core math (C=512, L=2048, G=4 groups, NH=4 heads, HD=128):
  h  = groupnorm(x)                    (group == one 128-partition tile)
  q  = wq @ h + bq   [d, l] layout     (PE-transposed weights)
  k  = wk @ h + bk   [d, l]
  vT = h^T @ wv^T + bv  [l, d] layout  (produced transposed; no V transposes)
  sT[k,q] = k_chunk^T q  -> exp (no max-sub; logits ~ N(0,1))
  den = ones^T exp (cross-partition sum, broadcast to 128 partitions)
  avT[d,q] = sum_kt vT_chunk^T exp_chunk ; attn = avT * (1/den)
  out = wo @ attn + bo + x

Matmuls run as float32r (full-rate fp32 mode). fp32r is a distinct lossy bit
layout (~2^-12 relative); every fp32r operand is produced by a compute engine
writing dtype float32r (conversions folded into PSUM->SBUF moves that exist
anyway; those moves run on the scalar engine to keep the vector engine off
the PSUM-drain critical path).

Scheduling: weight loads + PE transposes are emitted first so the tensor
engine has work while groupnorm stats stream in; the attention inner loop is
software-pipelined per k-tile (QK+exp of tile i+1 ahead of den/av of tile i);
the attention loop is q-chunk-outer and the output projection for each
l-chunk uses a dedicated PSUM slot so it overlaps the next q-chunk's
attention.

PSUM budget (8 banks): s4 3x1 + den 2x1 + av 2x1 + op 1x1.
"""

import os
import numpy as np

import concourse.bass as bass
import concourse.tile as tile
from concourse import bacc, mybir
from concourse.bass_utils import run_bass_kernel_spmd
from concourse.masks import make_identity

F32 = mybir.dt.float32
F32R = mybir.dt.float32r

B, C, L = 8, 512, 2048
G = 4            # groupnorm groups; group size 128 == one partition tile
NH, HD = 4, 128  # heads, head dim
CT = C // 128    # 4 channel tiles
LC = L // 512    # 4 l-chunks of 512
LT = L // 128    # 16 l-tiles of 128
EPS = 1e-6
SM_SCALE = float(HD) ** -0.5

AFT = mybir.ActivationFunctionType
ALU = mybir.AluOpType


def build_attn_block(nc):
    x_d = nc.dram_tensor("x", [C, L], F32, kind="ExternalInput").ap()
    gs_d = nc.dram_tensor("gn_scale", [C], F32, kind="ExternalInput").ap()
    gb_d = nc.dram_tensor("gn_bias", [C], F32, kind="ExternalInput").ap()
    w_d = {}
    b_d = {}
    for nm in ("q", "k", "v", "o"):
        w_d[nm] = nc.dram_tensor(f"w{nm}", [C, C], F32, kind="ExternalInput").ap()
        b_d[nm] = nc.dram_tensor(f"b{nm}", [C], F32, kind="ExternalInput").ap()
    out_d = nc.dram_tensor("out", [C, L], F32, kind="ExternalOutput").ap()

    with tile.TileContext(nc) as tc:
        with (
            tc.tile_pool(name="const", bufs=1) as const,
            tc.tile_pool(name="wstage", bufs=2) as wstage,
            tc.tile_pool(name="wt", bufs=1) as wt,
            tc.tile_pool(name="big", bufs=1) as big,
            tc.tile_pool(name="small", bufs=4) as small,
            tc.tile_pool(name="epool", bufs=4) as epool,
            tc.tile_pool(name="cpool", bufs=2) as cpool,
            tc.tile_pool(name="psum", bufs=2, space="PSUM") as psum,
        ):
            # ---- constants ----
            identity = const.tile([128, 128], F32)
            make_identity(nc, identity)
            ones = const.tile([128, 128], F32)
            nc.vector.memset(ones, 1.0)
            ones_r = const.tile([128, 128], F32R)
            nc.vector.tensor_copy(ones_r, ones)
            eps_t = const.tile([128, 1], F32)
            nc.vector.memset(eps_t, EPS)

            def load_cvec(name, ap_1d):
                t = const.tile([128, CT], F32, name=name)
                nc.sync.dma_start(out=t, in_=ap_1d.rearrange("(t p) -> p t", p=128))
                return t

            bq_sb = load_cvec("bq_sb", b_d["q"])
            bk_sb = load_cvec("bk_sb", b_d["k"])
            bo_sb = load_cvec("bo_sb", b_d["o"])
            gs_sb = load_cvec("gs_sb", gs_d)
            gb_sb = load_cvec("gb_sb", gb_d)

            bv_bc = cpool.tile([128, C], F32, tag="ot_sb")  # bv broadcast
            nc.sync.dma_start(
                out=bv_bc,
                in_=bass.AP(
                    tensor=b_d["v"].tensor,
                    offset=b_d["v"].offset,
                    ap=[[0, 128]] + list(b_d["v"].ap),
                ),
            )

            # ---- weights: load row-blocks + PE-transpose (plain fp32) into
            #      wT[c, o], converting to fp32r in the PSUM->SBUF copy.
            #      Emitted first: gives the PE work while groupnorm streams x.
            wts = {}
            for nm in ("q", "k", "v", "o"):
                wts[nm] = wt.tile([128, CT, C], F32R, name=f"w{nm}t")
            pt_tags = [("s4", 3), ("den", 2), ("av", 2), ("s4", 3),
                       ("den", 2), ("av", 2), ("op", 1)]
            ti = 0
            wblocks = [(nm, ot) for nm in ("v", "q", "k", "o") for ot in range(CT)]
            wbi = [0]

            def emit_weight_blocks(n):
                nonlocal ti
                for _ in range(n):
                    if wbi[0] >= len(wblocks):
                        return
                    nm, ot = wblocks[wbi[0]]
                    wbi[0] += 1
                    stg = wstage.tile([128, C], F32, tag="stg")
                    nc.sync.dma_start(
                        out=stg, in_=w_d[nm][ot * 128 : (ot + 1) * 128, :]
                    )
                    for ct in range(CT):
                        tag, tb = pt_tags[ti % len(pt_tags)]
                        pt = psum.tile([128, 128], F32, tag=tag, bufs=tb, name="pt")
                        nc.tensor.transpose(
                            pt, stg[:, ct * 128 : (ct + 1) * 128], identity
                        )
                        dstw = wts[nm][:, ct, ot * 128 : (ot + 1) * 128]
                        if ti % 2 == 0:
                            nc.scalar.copy(dstw, pt)
                        else:
                            nc.vector.tensor_copy(dstw, pt)
                        ti += 1

            # ---- groupnorm stats: stream x in [128,1024] chunks ----
            x_r = x_d.rearrange("(t p) l -> p t l", p=128)
            h_sb = big.tile([128, CT, L], F32R, tag="xattn")
            gn_ab = []  # (a_t, b_t) per channel tile
            for ct in range(CT):
                stats = small.tile([128, 4, 6], F32, tag="stats")
                for i2 in range(2):
                    xc = cpool.tile([128, 1024], F32, tag="xc", bufs=2)
                    nc.sync.dma_start(
                        out=xc, in_=x_r[:, ct, i2 * 1024 : (i2 + 1) * 1024]
                    )
                    for j in range(2):
                        i = i2 * 2 + j
                        nc.vector.bn_stats(
                            out=stats[:, i, :], in_=xc[:, j * 512 : (j + 1) * 512]
                        )
                mv = small.tile([128, 2], F32, tag="mv")
                nc.vector.bn_aggr(out=mv, in_=stats)
                # stat2 = [mean_p, E[x^2]_p]
                stat2 = small.tile([128, 2], F32, tag="stat2")
                nc.vector.tensor_copy(stat2[:, 0:1], mv[:, 0:1])
                nc.vector.scalar_tensor_tensor(
                    out=stat2[:, 1:2],
                    in0=mv[:, 0:1],
                    scalar=mv[:, 0:1],
                    in1=mv[:, 1:2],
                    op0=ALU.mult,
                    op1=ALU.add,
                )
                pg = psum.tile([128, 2], F32, tag="den")
                nc.tensor.matmul(pg, ones, stat2, start=True, stop=True)
                mean_t = small.tile([128, 1], F32, tag="mean_t")
                nc.vector.tensor_scalar_mul(mean_t, pg[:, 0:1], 1.0 / 128.0)
                ex2_t = small.tile([128, 1], F32, tag="ex2_t")
                nc.vector.tensor_scalar_mul(ex2_t, pg[:, 1:2], 1.0 / 128.0)
                var_t = small.tile([128, 1], F32, tag="var_t")
                nc.vector.tensor_mul(var_t, mean_t, mean_t)
                nc.vector.tensor_sub(var_t, ex2_t, var_t)
                std_t = small.tile([128, 1], F32, tag="std_t")
                nc.scalar.activation(std_t, var_t, AFT.Sqrt, bias=eps_t)
                rstd_t = small.tile([128, 1], F32, tag="rstd_t")
                nc.vector.reciprocal(rstd_t, std_t)
                a_t = small.tile([128, 1], F32, tag="a_t", bufs=CT)
                nc.vector.tensor_mul(a_t, rstd_t, gs_sb[:, ct : ct + 1])
                b_t = small.tile([128, 1], F32, tag="b_t", bufs=CT)
                nc.vector.tensor_mul(b_t, mean_t, a_t)
                nc.vector.tensor_sub(b_t, gb_sb[:, ct : ct + 1], b_t)
                gn_ab.append((a_t, b_t))
                emit_weight_blocks(4)

            # ---- groupnorm apply: h = a*x + b, written as fp32r.
            #      l-chunk outer so early l-chunks of h complete first. ----
            emit_weight_blocks(len(wblocks))
            for l2 in range(2):
                for ct in range(CT):
                    a_t, b_t = gn_ab[ct]
                    xc = cpool.tile([128, 1024], F32, tag="xc", bufs=2)
                    nc.sync.dma_start(
                        out=xc, in_=x_r[:, ct, l2 * 1024 : (l2 + 1) * 1024]
                    )
                    nc.scalar.activation(
                        h_sb[:, ct, l2 * 1024 : (l2 + 1) * 1024],
                        xc,
                        AFT.Identity,
                        bias=b_t,
                        scale=a_t,
                    )

            # ---- vT projection first (attention needs all of it) ----
            vT_sb = big.tile([128, LT, C], F32R, tag="vT_sb")
            for lt in range(LT):
                pp = psum.tile([128, 512], F32, tag="den")
                for ct in range(CT):
                    nc.tensor.matmul(
                        pp,
                        h_sb[:, ct, lt * 128 : (lt + 1) * 128],
                        wts["v"][:, ct, :],
                        start=(ct == 0),
                        stop=(ct == CT - 1),
                    )
                nc.vector.tensor_add(vT_sb[:, lt, :], pp, bv_bc)

            # ---- q, k projections: [d, l], head-major; bias-add + fp32r
            #      conversion on the scalar engine ----
            q_sb = big.tile([128, NH, L], F32R, tag="q_sb")
            k_sb = big.tile([128, NH, L], F32R, tag="k_sb")
            for h in range(NH):
                for dst, wtt, bias in (
                    (q_sb, wts["q"], bq_sb),
                    (k_sb, wts["k"], bk_sb),
                ):
                    for lc in range(LC):
                        pp = psum.tile([128, 512], F32, tag="av")
                        for ct in range(CT):
                            nc.tensor.matmul(
                                pp,
                                wtt[:, ct, h * 128 : (h + 1) * 128],
                                h_sb[:, ct, lc * 512 : (lc + 1) * 512],
                                start=(ct == 0),
                                stop=(ct == CT - 1),
                            )
                        nc.scalar.activation(
                            dst[:, h, lc * 512 : (lc + 1) * 512],
                            pp,
                            AFT.Identity,
                            bias=bias[:, h : h + 1],
                        )

            # ---- attention (q-chunk outer), software-pipelined per k-tile;
            #      out-projection per l-chunk overlaps the next q-chunk ----
            attn_sb = big.tile([128, NH, L], F32R, tag="xattn")

            def emit_qk_exp(h, qc, kt):
                ps = psum.tile([128, 512], F32, tag="s4", bufs=3)
                nc.tensor.matmul(
                    ps,
                    k_sb[:, h, kt * 128 : (kt + 1) * 128],
                    q_sb[:, h, qc * 512 : (qc + 1) * 512],
                    start=True,
                    stop=True,
                )
                e2 = epool.tile([128, 512], F32R, tag="e2", bufs=5)
                nc.scalar.activation(e2, ps, AFT.Exp, scale=SM_SCALE)
                return e2

            def emit_den_av(h, qc, kt, e2, pden, pav):
                nc.tensor.matmul(
                    pden, ones_r, e2, start=(kt == 0), stop=(kt == LT - 1)
                )
                nc.tensor.matmul(
                    pav,
                    vT_sb[:, kt, h * 128 : (h + 1) * 128],
                    e2,
                    start=(kt == 0),
                    stop=(kt == LT - 1),
                )

            def finish_chunk(h, qc, pden, pav):
                rden = cpool.tile([128, 512], F32, tag="rden", bufs=1, name="rden")
                nc.vector.reciprocal(rden, pden)
                nc.vector.tensor_mul(
                    attn_sb[:, h, qc * 512 : (qc + 1) * 512], pav, rden
                )

            def emit_out_proj(lc, last):
                for ot in range(CT):
                    xr = cpool.tile([128, 512], F32, tag="xc", bufs=2, name="xr")
                    nc.sync.dma_start(
                        out=xr,
                        in_=x_d[
                            ot * 128 : (ot + 1) * 128, lc * 512 : (lc + 1) * 512
                        ],
                    )
                    # the final l-chunk may use the attention "den" slots
                    # (attention is over by then) for 2-deep overlap
                    pp = (
                        psum.tile([128, 512], F32, tag="den", name="pp")
                        if last
                        else psum.tile([128, 512], F32, tag="op", bufs=1, name="pp")
                    )
                    for ct in range(CT):
                        nc.tensor.matmul(
                            pp,
                            wts["o"][:, ct, ot * 128 : (ot + 1) * 128],
                            attn_sb[:, ct, lc * 512 : (lc + 1) * 512],
                            start=(ct == 0),
                            stop=(ct == CT - 1),
                        )
                    ot_sb = cpool.tile([128, 512], F32, tag="ot_sb")
                    nc.vector.scalar_tensor_tensor(
                        out=ot_sb,
                        in0=pp,
                        scalar=bo_sb[:, ot : ot + 1],
                        in1=xr,
                        op0=ALU.add,
                        op1=ALU.add,
                    )
                    nc.sync.dma_start(
                        out=out_d[
                            ot * 128 : (ot + 1) * 128, lc * 512 : (lc + 1) * 512
                        ],
                        in_=ot_sb,
                    )

            DEPTH = 3  # den/av lag QK+exp by this many k-tiles

            def drain_one(pq):
                p = pq.pop(0)
                emit_den_av(*p)
                if p[2] == LT - 1:
                    finish_chunk(p[0], p[1], p[4], p[5])

            deferred_out = None  # l-chunk whose out-projection awaits emission
            for qc in range(LC):
                pipeline = []
                for h in range(NH):
                    pden = psum.tile([128, 512], F32, tag="den")
                    pav = psum.tile([128, 512], F32, tag="av")
                    for kt in range(LT):
                        e2 = emit_qk_exp(h, qc, kt)
                        if len(pipeline) >= DEPTH:
                            drain_one(pipeline)
                        pipeline.append((h, qc, kt, e2, pden, pav))
                        # emit the previous q-chunk's out-projection a few
                        # k-tiles into this one, so the PE queue has ready
                        # attention work while that chain completes
                        if deferred_out is not None and h == 0 and kt == 6:
                            emit_out_proj(deferred_out, last=False)
                            deferred_out = None
                # flush so the out-projection sees completed attention columns
                while pipeline:
                    drain_one(pipeline)
                deferred_out = qc
            emit_out_proj(deferred_out, last=True)
    nc.compile()
    return nc


_NC_CACHE = {}


def _get_nc():
    if "nc" not in _NC_CACHE:
        nc = bacc.Bacc("TRN2", debug=False)
        build_attn_block(nc)
        _NC_CACHE["nc"] = nc
    return _NC_CACHE["nc"]


def run(trace=False, **inputs):
    nc = _get_nc()
    xs = np.ascontiguousarray(np.asarray(inputs["x"], dtype=np.float32))
    shared = {}
    for nm in ("gn_scale", "gn_bias", "wq", "bq", "wk", "bk", "wv", "bv", "wo", "bo"):
        shared[nm] = np.ascontiguousarray(np.asarray(inputs[nm], dtype=np.float32))
    in_maps = [dict(shared, x=xs[b]) for b in range(B)]
    res = run_bass_kernel_spmd(nc, in_maps, core_ids=list(range(B)), trace=trace)
    out = np.stack([res.results[b]["out"] for b in range(B)], axis=0)
    return out, res


def kernel(**inputs):
    out, _ = run(trace=bool(os.environ.get("ATTN_TRACE")), **inputs)
    return out



# revision 5
# speedup vs baseline: 1.2444x; 1.2444x over previous
"""AttnBlock (GroupNorm + 4-head d=128 self-attention + residual).

Full input x: [8, 512, 2048] fp32. Data-parallel over batch: core b computes
batch b entirely on-chip (no collectives).

Per-core math (C=512, L=2048, G=4 groups, NH=4 heads, HD=128):
  h  = groupnorm(x)          bf16, [128, CT, L]; x stays resident fp32
  q  = wq @ h + bq           bf16 [d, l] head-major (PE-transposed weights)
  k  = wk @ h + bk           bf16 [d, l]
  vT = h^T @ wv^T + bv       bf16 [l, d] (produced transposed)
  sT[k,q] = k_tile^T q       fp32 PSUM, two k-tiles per 2-bank group
  e = exp(s * scale)         one ACT instr per [128,1024] group -> bf16
  den: DVE bf16 pairwise-add tree over the 8 e-groups -> [128,512],
       then one ones-matmul broadcasts the cross-partition sum
  rden = reciprocal_approx_fast(den)   (DVE custom op, ~51 ULP)
  attn = pav * rden          bf16
  out = wo @ attn + bo + x   (STT drain fuses bias + residual)

All matmul operands are bf16 (fp32 PSUM accumulation); tolerance is 2e-2
rel-L2 and bf16 lands ~1e-3. Engine split aims to balance PE (~170us:
qk/av/projections), ACT (~160us: exp stream + groupnorm apply), and DVE
(~165us: den tree, drains, stats).

Scheduling: weight loads + PE transposes are emitted first and interleaved
with the x stream + groupnorm stats; the exp table set is preloaded with a
dummy activation; attention is qc-outer/head-inner with scores double-
buffered across two 2-bank PSUM groups; v projection is emitted just-in-time
inside unit (0,0); k/q projections for head h are emitted right before that
head's first unit; the out-projection for qc is deferred into qc+1's first
unit.

PSUM budget (8 banks): sA 2 + sB 2 + av 1 + dn 1 + pp 2.
"""

import os
import numpy as np

import concourse.bass as bass
import concourse.tile as tile
from concourse import bacc, mybir
from concourse.bass_utils import run_bass_kernel_spmd
from concourse.masks import make_identity

F32 = mybir.dt.float32
BF16 = mybir.dt.bfloat16

B, C, L = 8, 512, 2048
G = 4            # groupnorm groups; group size 128 == one partition tile
NH, HD = 4, 128  # heads, head dim
CT = C // 128    # 4 channel tiles
LC = L // 512    # 4 l-chunks of 512
LT = L // 128    # 16 l-tiles of 128
NG = LT // 2     # 8 score groups of 2 k-tiles
EPS = 1e-6
SM_SCALE = float(HD) ** -0.5

AFT = mybir.ActivationFunctionType
ALU = mybir.AluOpType


def build_attn_block(nc):
    x_d = nc.dram_tensor("x", [C, L], F32, kind="ExternalInput").ap()
    gs_d = nc.dram_tensor("gn_scale", [C], F32, kind="ExternalInput").ap()
    gb_d = nc.dram_tensor("gn_bias", [C], F32, kind="ExternalInput").ap()
    w_d = {}
    b_d = {}
    for nm in ("q", "k", "v", "o"):
        w_d[nm] = nc.dram_tensor(f"w{nm}", [C, C], F32, kind="ExternalInput").ap()
        b_d[nm] = nc.dram_tensor(f"b{nm}", [C], F32, kind="ExternalInput").ap()
    out_d = nc.dram_tensor("out", [C, L], F32, kind="ExternalOutput").ap()

    with tile.TileContext(nc) as tc:
        with (
            tc.tile_pool(name="const", bufs=1) as const,
            tc.tile_pool(name="wstage", bufs=2) as wstage,
            tc.tile_pool(name="wt", bufs=1) as wt,
            tc.tile_pool(name="big", bufs=1) as big,
            tc.tile_pool(name="small", bufs=4) as small,
            tc.tile_pool(name="epool", bufs=4) as epool,
            tc.tile_pool(name="tpool", bufs=6) as tpool,
            tc.tile_pool(name="cpool", bufs=2) as cpool,
            tc.tile_pool(name="psum", bufs=1, space="PSUM") as psum,
        ):
            # ---- constants ----
            identity = const.tile([128, 128], F32)
            make_identity(nc, identity)
            ones = const.tile([128, 128], F32)
            nc.vector.memset(ones, 1.0)
            ones_bf = const.tile([128, 128], BF16)
            nc.vector.tensor_copy(ones_bf, ones)
            eps_t = const.tile([128, 1], F32)
            nc.vector.memset(eps_t, EPS)

            def load_cvec(name, ap_1d):
                t = const.tile([128, CT], F32, name=name)
                nc.sync.dma_start(out=t, in_=ap_1d.rearrange("(t p) -> p t", p=128))
                return t

            bq_sb = load_cvec("bq_sb", b_d["q"])
            bk_sb = load_cvec("bk_sb", b_d["k"])
            bo_sb = load_cvec("bo_sb", b_d["o"])
            gs_sb = load_cvec("gs_sb", gs_d)
            gb_sb = load_cvec("gb_sb", gb_d)

            bv_bc = const.tile([128, C], F32)  # bv broadcast across partitions
            nc.sync.dma_start(
                out=bv_bc,
                in_=bass.AP(
                    tensor=b_d["v"].tensor,
                    offset=b_d["v"].offset,
                    ap=[[0, 128]] + list(b_d["v"].ap),
                ),
            )

            # ---- big persistent tiles ----
            x_sb = big.tile([128, CT, L], F32, tag="x_sb")
            h_sb = big.tile([128, CT, L], BF16, tag="h_sb")
            q_sb = big.tile([128, NH, L], BF16, tag="q_sb")
            k_sb = big.tile([128, NH, L], BF16, tag="k_sb")
            vT_sb = big.tile([128, LT, C], BF16, tag="vT_sb")
            attn_sb = big.tile([128, NH, L], BF16, tag="attn_sb")

            # ---- weights: DMA row-blocks, PE-transpose into wT[c_in, c_out]
            #      (bf16), drained as one [128, 4, 128] strided copy per block.
            #      Order: wk0, wq0 (head-0 lead-in), wv (v proj), rest.
            wts = {}
            for nm in ("q", "k", "v", "o"):
                wts[nm] = wt.tile([128, CT, C], BF16, name=f"w{nm}t")
            wblocks = (
                [("k", 0), ("q", 0)]
                + [("v", ot) for ot in range(CT)]
                + [("k", 1), ("q", 1), ("k", 2), ("q", 2), ("k", 3), ("q", 3)]
                + [("o", ot) for ot in range(CT)]
            )
            wbi = [0]
            drain_flip = [0]

            def emit_weight_blocks(n):
                for _ in range(n):
                    if wbi[0] >= len(wblocks):
                        return
                    nm, ot = wblocks[wbi[0]]
                    wbi[0] += 1
                    stg = wstage.tile([128, C], F32, tag="stg")
                    nc.sync.dma_start(
                        out=stg, in_=w_d[nm][ot * 128 : (ot + 1) * 128, :]
                    )
                    pt = psum.tile([128, 512], F32, tag="pp", bufs=2, name="pt")
                    for ct in range(CT):
                        nc.tensor.transpose(
                            pt[:, ct * 128 : (ct + 1) * 128],
                            stg[:, ct * 128 : (ct + 1) * 128],
                            identity,
                        )
                    dstw = wts[nm][:, :, ot * 128 : (ot + 1) * 128]
                    if drain_flip[0] % 2 == 0:
                        nc.vector.tensor_copy(dstw, pt.rearrange("p (c t) -> p c t", c=CT))
                    else:
                        nc.scalar.copy(dstw, pt.rearrange("p (c t) -> p c t", c=CT))
                    drain_flip[0] += 1

            # ---- x stream (ct-major) + groupnorm stats ----
            x_r = x_d.rearrange("(t p) l -> p t l", p=128)
            gn_ab = []
            for ct in range(CT):
                for i2 in range(2):
                    nc.sync.dma_start(
                        out=x_sb[:, ct, i2 * 1024 : (i2 + 1) * 1024],
                        in_=x_r[:, ct, i2 * 1024 : (i2 + 1) * 1024],
                    )
                stats = small.tile([128, 4, 6], F32, tag="stats")
                for i in range(4):
                    nc.vector.bn_stats(
                        out=stats[:, i, :], in_=x_sb[:, ct, i * 512 : (i + 1) * 512]
                    )
                mv = small.tile([128, 2], F32, tag="mv")
                nc.vector.bn_aggr(out=mv, in_=stats)
                stat2 = small.tile([128, 2], F32, tag="stat2")
                nc.vector.tensor_copy(stat2[:, 0:1], mv[:, 0:1])
                nc.vector.scalar_tensor_tensor(
                    out=stat2[:, 1:2],
                    in0=mv[:, 0:1],
                    scalar=mv[:, 0:1],
                    in1=mv[:, 1:2],
                    op0=ALU.mult,
                    op1=ALU.add,
                )
                pg = psum.tile([128, 2], F32, tag="pp", bufs=2, name="pg")
                nc.tensor.matmul(pg, ones, stat2, start=True, stop=True)
                mean_t = small.tile([128, 1], F32, tag="mean_t")
                nc.vector.tensor_scalar_mul(mean_t, pg[:, 0:1], 1.0 / 128.0)
                ex2_t = small.tile([128, 1], F32, tag="ex2_t")
                nc.vector.tensor_scalar_mul(ex2_t, pg[:, 1:2], 1.0 / 128.0)
                var_t = small.tile([128, 1], F32, tag="var_t")
                nc.vector.tensor_mul(var_t, mean_t, mean_t)
                nc.vector.tensor_sub(var_t, ex2_t, var_t)
                std_t = small.tile([128, 1], F32, tag="std_t")
                nc.scalar.activation(std_t, var_t, AFT.Sqrt, bias=eps_t)
                rstd_t = small.tile([128, 1], F32, tag="rstd_t")
                nc.vector.reciprocal(rstd_t, std_t)
                a_t = small.tile([128, 1], F32, tag="a_t", bufs=CT)
                nc.vector.tensor_mul(a_t, rstd_t, gs_sb[:, ct : ct + 1])
                b_t = small.tile([128, 1], F32, tag="b_t", bufs=CT)
                nc.vector.tensor_mul(b_t, mean_t, a_t)
                nc.vector.tensor_sub(b_t, gb_sb[:, ct : ct + 1], b_t)
                gn_ab.append((a_t, b_t))
                emit_weight_blocks(2)

            # preload the exp table set while the tail of x streams in
            dummy = small.tile([128, 1], F32, tag="dummy")
            nc.scalar.activation(dummy, eps_t, AFT.Exp)

            # ---- groupnorm apply: one big ACT instr per channel tile ----
            for ct in range(CT):
                a_t, b_t = gn_ab[ct]
                nc.scalar.activation(
                    h_sb[:, ct, :], x_sb[:, ct, :], AFT.Identity, bias=b_t, scale=a_t
                )
            emit_weight_blocks(len(wblocks))

            # ---- projection helpers ----
            def emit_kq_proj(h):
                for dst, wtt, bias in (
                    (k_sb, wts["k"], bk_sb),
                    (q_sb, wts["q"], bq_sb),
                ):
                    for lc in range(LC):
                        pp = psum.tile([128, 512], F32, tag="pp", bufs=2, name="pp")
                        for ct in range(CT):
                            nc.tensor.matmul(
                                pp,
                                wtt[:, ct, h * 128 : (h + 1) * 128],
                                h_sb[:, ct, lc * 512 : (lc + 1) * 512],
                                start=(ct == 0),
                                stop=(ct == CT - 1),
                            )
                        nc.vector.tensor_scalar_add(
                            dst[:, h, lc * 512 : (lc + 1) * 512],
                            pp,
                            bias[:, h : h + 1],
                        )

            def emit_v_tiles(lt0, n):
                for lt in range(lt0, lt0 + n):
                    pv = psum.tile([128, 512], F32, tag="pp", bufs=2, name="pv")
                    for ct in range(CT):
                        nc.tensor.matmul(
                            pv,
                            h_sb[:, ct, lt * 128 : (lt + 1) * 128],
                            wts["v"][:, ct, :],
                            start=(ct == 0),
                            stop=(ct == CT - 1),
                        )
                    nc.vector.tensor_add(vT_sb[:, lt, :], pv, bv_bc)

            def emit_out_proj_ot(qc, ot):
                pop = psum.tile([128, 512], F32, tag="pp", bufs=2, name="pop")
                for ct in range(CT):
                    nc.tensor.matmul(
                        pop,
                        wts["o"][:, ct, ot * 128 : (ot + 1) * 128],
                        attn_sb[:, ct, qc * 512 : (qc + 1) * 512],
                        start=(ct == 0),
                        stop=(ct == CT - 1),
                    )
                ot_sb = cpool.tile([128, 512], F32, tag="ot_sb")
                nc.vector.scalar_tensor_tensor(
                    out=ot_sb,
                    in0=pop,
                    scalar=bo_sb[:, ot : ot + 1],
                    in1=x_sb[:, ot, qc * 512 : (qc + 1) * 512],
                    op0=ALU.add,
                    op1=ALU.add,
                )
                nc.sync.dma_start(
                    out=out_d[ot * 128 : (ot + 1) * 128, qc * 512 : (qc + 1) * 512],
                    in_=ot_sb,
                )

            def emit_out_proj(qc):
                for ot in range(CT):
                    emit_out_proj_ot(qc, ot)

            # ---- attention ----
            def emit_qk(h, qc, g):
                ps = psum.tile(
                    [128, 1024], F32, tag=("sA" if g % 2 == 0 else "sB"), name="ps"
                )
                for j in range(2):
                    kt = 2 * g + j
                    nc.tensor.matmul(
                        ps[:, j * 512 : (j + 1) * 512],
                        k_sb[:, h, kt * 128 : (kt + 1) * 128],
                        q_sb[:, h, qc * 512 : (qc + 1) * 512],
                        start=True,
                        stop=True,
                    )
                e = epool.tile([128, 1024], BF16, tag="e", bufs=4, name="e")
                nc.scalar.activation(e, ps, AFT.Exp, scale=SM_SCALE)
                return e

            def emit_av(h, pav, e, g):
                for j in range(2):
                    kt = 2 * g + j
                    nc.tensor.matmul(
                        pav,
                        vT_sb[:, kt, h * 128 : (h + 1) * 128],
                        e[:, j * 512 : (j + 1) * 512],
                        start=(kt == 0),
                        stop=(kt == LT - 1),
                    )

            def finish_unit(st):
                # den broadcast + 1/den + softmax-normalize for a completed
                # unit; deferred into the NEXT unit so the PE never waits on
                # the DVE tree.
                h, qc, pav, f = st
                pden = psum.tile([128, 512], F32, tag="pp", bufs=2, name="pden")
                nc.tensor.matmul(pden, ones_bf, f, start=True, stop=True)
                rden = cpool.tile([128, 512], F32, tag="rden", name="rden")
                nc.vector.reciprocal_approx_fast(rden, pden)
                nc.vector.tensor_mul(
                    attn_sb[:, h, qc * 512 : (qc + 1) * 512], pav, rden
                )

            def emit_unit(h, qc, hook=None):
                # software-pipelined: qk(g)/exp(g) one group ahead of av(g);
                # DVE pair-adds trail; returns state for the deferred finish.
                es = []
                ts = []
                pav = psum.tile([128, 512], F32, tag="av", bufs=2, name="pav")
                es.append(emit_qk(h, qc, 0))
                for g in range(1, NG + 1):
                    if g < NG:
                        es.append(emit_qk(h, qc, g))
                    if hook is not None:
                        hook(g)
                    emit_av(h, pav, es[g - 1], g - 1)
                    if g % 2 == 0:
                        t = tpool.tile([128, 1024], BF16, tag="t", bufs=6, name="t")
                        nc.vector.tensor_add(t, es[g - 2], es[g - 1])
                        ts.append(t)
                        if g == 4:
                            u0 = tpool.tile(
                                [128, 1024], BF16, tag="t", bufs=6, name="u0"
                            )
                            nc.vector.tensor_add(u0, ts[0], ts[1])
                if len(ts) != 4:
                    raise AssertionError("expected 4 pair sums")
                u1 = tpool.tile([128, 1024], BF16, tag="t", bufs=6, name="u1")
                nc.vector.tensor_add(u1, ts[2], ts[3])
                s = tpool.tile([128, 1024], BF16, tag="t", bufs=6, name="s")
                nc.vector.tensor_add(s, u0, u1)
                f = tpool.tile([128, 512], BF16, tag="f", bufs=2, name="f")
                nc.vector.tensor_add(f, s[:, 0:512], s[:, 512:1024])
                return (h, qc, pav, f)

            emit_kq_proj(0)
            pending = None  # completed unit awaiting den/normalize
            deferred_out = None  # qc whose out-projection awaits emission
            for qc in range(LC):
                for h in range(NH):
                    dq = deferred_out if h == 0 and qc > 0 else None
                    if h == 0 and qc > 0:
                        deferred_out = None
                    vjit = qc == 0 and h == 0
                    pend = pending

                    def hook(g, _dq=dq, _vjit=vjit, _pend=pend):
                        if _vjit:
                            emit_v_tiles(2 * (g - 1), 2)
                        if g == 2 and _pend is not None:
                            finish_unit(_pend)
                        if _dq is not None and 3 <= g <= 6:
                            emit_out_proj_ot(_dq, g - 3)

                    pending = emit_unit(h, qc, hook)
                    if qc == 0 and h < NH - 1:
                        emit_kq_proj(h + 1)
                deferred_out = qc
            finish_unit(pending)
            emit_out_proj(deferred_out)
    nc.compile()
    return nc


_NC_CACHE = {}


def _get_nc():
    if "nc" not in _NC_CACHE:
        nc = bacc.Bacc("TRN2", debug=False)
        build_attn_block(nc)
        _NC_CACHE["nc"] = nc
    return _NC_CACHE["nc"]


def run(trace=False, **inputs):
    nc = _get_nc()
    xs = np.ascontiguousarray(np.asarray(inputs["x"], dtype=np.float32))
    shared = {}
    for nm in ("gn_scale", "gn_bias", "wq", "bq", "wk", "bk", "wv", "bv", "wo", "bo"):
        shared[nm] = np.ascontiguousarray(np.asarray(inputs[nm], dtype=np.float32))
    in_maps = [dict(shared, x=xs[b]) for b in range(B)]
    res = run_bass_kernel_spmd(nc, in_maps, core_ids=list(range(B)), trace=trace)
    out = np.stack([res.results[b]["out"] for b in range(B)], axis=0)
    return out, res


def kernel(**inputs):
    out, _ = run(trace=bool(os.environ.get("ATTN_TRACE")), **inputs)
    return out


# revision 8
# speedup vs baseline: 1.2604x; 1.0128x over previous
"""AttnBlock (GroupNorm + 4-head d=128 self-attention + residual).

Full input x: [8, 512, 2048] fp32. Data-parallel over batch: core b computes
batch b entirely on-chip (no collectives).

Per-core math (C=512, L=2048, G=4 groups, NH=4 heads, HD=128):
  h  = groupnorm(x)          bf16; x stays resident fp32 (residual reuse)
  q  = wq @ h + bq           bf16 [d, l] head-major (PE-transposed weights)
  k  = wk @ h + bk           bf16 [d, l]
  vT = h^T @ wv^T + bv       bf16 [l, d]
  sT[k,q] = k_tile^T q       fp32 PSUM, two k-tiles per 2-bank group
  e = exp(s * scale)         one ACT instr per [128,1024] group -> bf16
  den: DVE bf16 pairwise-add tree over the 8 e-groups -> [128,512],
       one ones-matmul broadcasts the cross-partition sum
  rden = reciprocal_approx_fast(den)   (custom DVE op, ~51 ULP)
  attn = pav * rden          bf16
  out = wo @ attn + bo + x   (DVE STT fuses bias + residual)

All matmul operands bf16 (fp32 PSUM accumulation). Steady state is paced by
the ACT exp stream (~1.15us per [128,1024] group); PE rides just under it,
DVE (den tree + drains) well under.

Scheduling:
 - DMA order: wk0/wq0 row-blocks, then all of x, then wv, wk/wq rest, wo.
 - Groupnorm stats + apply run per channel-tile as x lands; k[0] and
   q[0,qc0] projections accumulate ct-by-ct in the (still unused)
   sA/sB/av PSUM banks so attention starts ~1us after h completes.
 - v projection is emitted just-in-time inside unit (0,0); head h+1's
   k/q projection rides unit (h,0)'s hooks; proj drains go on ACT
   (which idles during the PE-bound qc=0 region).
 - den/rden/normalize of unit U are deferred into unit U+1 (PE never
   waits on the DVE tree); out-projection of qc rides qc+1's first unit.

PSUM budget (8 banks): sA 2 + sB 2 + av 2 + pp 2.
"""

import os
import numpy as np

import concourse.bass as bass
import concourse.tile as tile
from concourse import bacc, mybir
from concourse.bass_utils import run_bass_kernel_spmd
from concourse.masks import make_identity

F32 = mybir.dt.float32
BF16 = mybir.dt.bfloat16

B, C, L = 8, 512, 2048
G = 4            # groupnorm groups; group size 128 == one partition tile
NH, HD = 4, 128  # heads, head dim
CT = C // 128    # 4 channel tiles
LC = L // 512    # 4 l-chunks of 512
LT = L // 128    # 16 l-tiles of 128
NG = LT // 2     # 8 score groups of 2 k-tiles
EPS = 1e-6
SM_SCALE = float(HD) ** -0.5

AFT = mybir.ActivationFunctionType
ALU = mybir.AluOpType


def build_attn_block(nc):
    x_d = nc.dram_tensor("x", [C, L], F32, kind="ExternalInput").ap()
    gs_d = nc.dram_tensor("gn_scale", [C], F32, kind="ExternalInput").ap()
    gb_d = nc.dram_tensor("gn_bias", [C], F32, kind="ExternalInput").ap()
    w_d = {}
    b_d = {}
    for nm in ("q", "k", "v", "o"):
        w_d[nm] = nc.dram_tensor(f"w{nm}", [C, C], F32, kind="ExternalInput").ap()
        b_d[nm] = nc.dram_tensor(f"b{nm}", [C], F32, kind="ExternalInput").ap()
    out_d = nc.dram_tensor("out", [C, L], F32, kind="ExternalOutput").ap()

    with tile.TileContext(nc) as tc:
        with (
            tc.tile_pool(name="const", bufs=1) as const,
            tc.tile_pool(name="wstage", bufs=2) as wstage,
            tc.tile_pool(name="wt", bufs=1) as wt,
            tc.tile_pool(name="big", bufs=1) as big,
            tc.tile_pool(name="small", bufs=4) as small,
            tc.tile_pool(name="epool", bufs=4) as epool,
            tc.tile_pool(name="tpool", bufs=6) as tpool,
            tc.tile_pool(name="cpool", bufs=2) as cpool,
            tc.tile_pool(name="psum", bufs=1, space="PSUM") as psum,
        ):
            # ---- constants ----
            identity = const.tile([128, 128], F32)
            make_identity(nc, identity)
            ones = const.tile([128, 128], F32)
            nc.vector.memset(ones, 1.0)
            ones_bf = const.tile([128, 128], BF16)
            nc.vector.tensor_copy(ones_bf, ones)
            eps_t = const.tile([128, 1], F32)
            nc.vector.memset(eps_t, EPS)

            def load_cvec(name, ap_1d):
                t = const.tile([128, CT], F32, name=name)
                nc.sync.dma_start(out=t, in_=ap_1d.rearrange("(t p) -> p t", p=128))
                return t

            bq_sb = load_cvec("bq_sb", b_d["q"])
            bk_sb = load_cvec("bk_sb", b_d["k"])
            bo_sb = load_cvec("bo_sb", b_d["o"])
            gs_sb = load_cvec("gs_sb", gs_d)
            gb_sb = load_cvec("gb_sb", gb_d)

            bv_bc = const.tile([128, C], F32)  # bv broadcast across partitions
            nc.sync.dma_start(
                out=bv_bc,
                in_=bass.AP(
                    tensor=b_d["v"].tensor,
                    offset=b_d["v"].offset,
                    ap=[[0, 128]] + list(b_d["v"].ap),
                ),
            )

            # ---- big persistent tiles ----
            x_sb = big.tile([128, CT, L], F32, tag="x_sb")
            h_sb = big.tile([128, CT, L], BF16, tag="h_sb")
            q_sb = big.tile([128, NH, L], BF16, tag="q_sb")
            k_sb = big.tile([128, NH, L], BF16, tag="k_sb")
            vT_sb = big.tile([128, LT, C], BF16, tag="vT_sb")
            attn_sb = big.tile([128, NH, L], BF16, tag="attn_sb")

            # ---- weights: DMA row-blocks, PE-transpose into wT[c_in, c_out]
            #      (bf16), drained as one [128, 4, 128] strided DVE copy.
            wts = {}
            for nm in ("q", "k", "v", "o"):
                wts[nm] = wt.tile([128, CT, C], BF16, name=f"w{nm}t")
            wblocks = (
                [("k", 0), ("q", 0)]
                + [("v", ot) for ot in range(CT)]
                + [("k", 1), ("q", 1), ("k", 2), ("q", 2), ("k", 3), ("q", 3)]
                + [("o", ot) for ot in range(CT)]
            )
            wbi = [0]

            def emit_weight_blocks(n):
                for _ in range(n):
                    if wbi[0] >= len(wblocks):
                        return
                    nm, ot = wblocks[wbi[0]]
                    wbi[0] += 1
                    stg = wstage.tile([128, C], F32, tag="stg")
                    nc.sync.dma_start(
                        out=stg, in_=w_d[nm][ot * 128 : (ot + 1) * 128, :]
                    )
                    pt = psum.tile([128, 512], F32, tag="pp", bufs=2, name="pt")
                    for ct in range(CT):
                        nc.tensor.transpose(
                            pt[:, ct * 128 : (ct + 1) * 128],
                            stg[:, ct * 128 : (ct + 1) * 128],
                            identity,
                        )
                    dstw = wts[nm][:, :, ot * 128 : (ot + 1) * 128]
                    nc.vector.tensor_copy(
                        dstw, pt.rearrange("p (c t) -> p c t", c=CT)
                    )

            x_r = x_d.rearrange("(t p) l -> p t l", p=128)

            # head-0 lead-in weights first, then x, then the rest
            emit_weight_blocks(2)
            for ct in range(CT):
                for i2 in range(2):
                    nc.sync.dma_start(
                        out=x_sb[:, ct, i2 * 1024 : (i2 + 1) * 1024],
                        in_=x_r[:, ct, i2 * 1024 : (i2 + 1) * 1024],
                    )

            # k[0] (4 lc) accumulates across the ct loop in the sA/sB banks;
            # q[0, lc0] in an av-tag bank. Attention has not started, so
            # those banks are free; pp stays free for stats pg + transposes.
            ps_k0 = [
                psum.tile([128, 1024], F32, tag="sA", name="ps_k0a"),
                psum.tile([128, 1024], F32, tag="sB", name="ps_k0b"),
            ]
            ps_q0 = psum.tile([128, 512], F32, tag="av", bufs=2, name="ps_q0")

            # ---- groupnorm stats + apply, per channel tile as x lands ----
            for ct in range(CT):
                stats = small.tile([128, 4, 6], F32, tag="stats")
                for i in range(4):
                    nc.vector.bn_stats(
                        out=stats[:, i, :], in_=x_sb[:, ct, i * 512 : (i + 1) * 512]
                    )
                mv = small.tile([128, 2], F32, tag="mv")
                nc.vector.bn_aggr(out=mv, in_=stats)
                stat2 = small.tile([128, 2], F32, tag="stat2")
                nc.vector.tensor_copy(stat2[:, 0:1], mv[:, 0:1])
                nc.vector.scalar_tensor_tensor(
                    out=stat2[:, 1:2],
                    in0=mv[:, 0:1],
                    scalar=mv[:, 0:1],
                    in1=mv[:, 1:2],
                    op0=ALU.mult,
                    op1=ALU.add,
                )
                pg = psum.tile([128, 2], F32, tag="pp", bufs=2, name="pg")
                nc.tensor.matmul(pg, ones, stat2, start=True, stop=True)
                mean_t = small.tile([128, 1], F32, tag="mean_t")
                nc.vector.tensor_scalar_mul(mean_t, pg[:, 0:1], 1.0 / 128.0)
                ex2_t = small.tile([128, 1], F32, tag="ex2_t")
                nc.vector.tensor_scalar_mul(ex2_t, pg[:, 1:2], 1.0 / 128.0)
                var_t = small.tile([128, 1], F32, tag="var_t")
                nc.vector.tensor_mul(var_t, mean_t, mean_t)
                nc.vector.tensor_sub(var_t, ex2_t, var_t)
                std_t = small.tile([128, 1], F32, tag="std_t")
                nc.scalar.activation(std_t, var_t, AFT.Sqrt, bias=eps_t)
                rstd_t = small.tile([128, 1], F32, tag="rstd_t")
                nc.vector.reciprocal(rstd_t, std_t)
                a_t = small.tile([128, 1], F32, tag="a_t", bufs=CT)
                nc.vector.tensor_mul(a_t, rstd_t, gs_sb[:, ct : ct + 1])
                b_t = small.tile([128, 1], F32, tag="b_t", bufs=CT)
                nc.vector.tensor_mul(b_t, mean_t, a_t)
                nc.vector.tensor_sub(b_t, gb_sb[:, ct : ct + 1], b_t)
                # h for this channel tile (ACT, one big instr)
                nc.scalar.activation(
                    h_sb[:, ct, :], x_sb[:, ct, :], AFT.Identity, bias=b_t, scale=a_t
                )
                # partial k[0] / q[0,lc0] accumulation on this channel tile
                for lc in range(LC):
                    nc.tensor.matmul(
                        ps_k0[lc // 2][:, (lc % 2) * 512 : (lc % 2 + 1) * 512],
                        wts["k"][:, ct, 0:128],
                        h_sb[:, ct, lc * 512 : (lc + 1) * 512],
                        start=(ct == 0),
                        stop=(ct == CT - 1),
                    )
                nc.tensor.matmul(
                    ps_q0,
                    wts["q"][:, ct, 0:128],
                    h_sb[:, ct, 0:512],
                    start=(ct == 0),
                    stop=(ct == CT - 1),
                )
                emit_weight_blocks(2 if ct < 2 else 3)

            # preload the exp table set before the attention stream begins
            dummy = small.tile([128, 1], F32, tag="dummy")
            nc.scalar.activation(dummy, eps_t, AFT.Exp)

            # drain k[0] / q[0,lc0] on ACT (DVE is busy with stats)
            for lc in range(LC):
                nc.scalar.activation(
                    k_sb[:, 0, lc * 512 : (lc + 1) * 512],
                    ps_k0[lc // 2][:, (lc % 2) * 512 : (lc % 2 + 1) * 512],
                    AFT.Identity,
                    bias=bk_sb[:, 0:1],
                )
            nc.scalar.activation(
                q_sb[:, 0, 0:512], ps_q0, AFT.Identity, bias=bq_sb[:, 0:1]
            )
            emit_weight_blocks(len(wblocks))

            # ---- projection helpers (drains on ACT: it idles during the
            #      PE-bound qc=0 region) ----
            def emit_proj_group(h, i):
                # i in 0..7: 0-3 -> k lc=i, 4-7 -> q lc=i-4
                dst, wtt, bias = (
                    (k_sb, wts["k"], bk_sb) if i < 4 else (q_sb, wts["q"], bq_sb)
                )
                lc = i % 4
                pp = psum.tile([128, 512], F32, tag="pp", bufs=2, name="pp")
                for ct in range(CT):
                    nc.tensor.matmul(
                        pp,
                        wtt[:, ct, h * 128 : (h + 1) * 128],
                        h_sb[:, ct, lc * 512 : (lc + 1) * 512],
                        start=(ct == 0),
                        stop=(ct == CT - 1),
                    )
                nc.scalar.activation(
                    dst[:, h, lc * 512 : (lc + 1) * 512],
                    pp,
                    AFT.Identity,
                    bias=bias[:, h : h + 1],
                )

            def emit_q0_group(lc):
                pp = psum.tile([128, 512], F32, tag="pp", bufs=2, name="pp")
                for ct in range(CT):
                    nc.tensor.matmul(
                        pp,
                        wts["q"][:, ct, 0:128],
                        h_sb[:, ct, lc * 512 : (lc + 1) * 512],
                        start=(ct == 0),
                        stop=(ct == CT - 1),
                    )
                nc.scalar.activation(
                    q_sb[:, 0, lc * 512 : (lc + 1) * 512],
                    pp,
                    AFT.Identity,
                    bias=bq_sb[:, 0:1],
                )

            def emit_v_tiles(lt0, n):
                for lt in range(lt0, lt0 + n):
                    pv = psum.tile([128, 512], F32, tag="pp", bufs=2, name="pv")
                    for ct in range(CT):
                        nc.tensor.matmul(
                            pv,
                            h_sb[:, ct, lt * 128 : (lt + 1) * 128],
                            wts["v"][:, ct, :],
                            start=(ct == 0),
                            stop=(ct == CT - 1),
                        )
                    nc.vector.tensor_add(vT_sb[:, lt, :], pv, bv_bc)

            def emit_out_proj_ot(qc, ot):
                pop = psum.tile([128, 512], F32, tag="pp", bufs=2, name="pop")
                for ct in range(CT):
                    nc.tensor.matmul(
                        pop,
                        wts["o"][:, ct, ot * 128 : (ot + 1) * 128],
                        attn_sb[:, ct, qc * 512 : (qc + 1) * 512],
                        start=(ct == 0),
                        stop=(ct == CT - 1),
                    )
                ot_sb = cpool.tile([128, 512], F32, tag="ot_sb")
                nc.vector.scalar_tensor_tensor(
                    out=ot_sb,
                    in0=pop,
                    scalar=bo_sb[:, ot : ot + 1],
                    in1=x_sb[:, ot, qc * 512 : (qc + 1) * 512],
                    op0=ALU.add,
                    op1=ALU.add,
                )
                nc.sync.dma_start(
                    out=out_d[ot * 128 : (ot + 1) * 128, qc * 512 : (qc + 1) * 512],
                    in_=ot_sb,
                )

            # ---- attention ----
            def emit_qk(h, qc, g):
                ps = psum.tile(
                    [128, 1024], F32, tag=("sA" if g % 2 == 0 else "sB"), name="ps"
                )
                for j in range(2):
                    kt = 2 * g + j
                    nc.tensor.matmul(
                        ps[:, j * 512 : (j + 1) * 512],
                        k_sb[:, h, kt * 128 : (kt + 1) * 128],
                        q_sb[:, h, qc * 512 : (qc + 1) * 512],
                        start=True,
                        stop=True,
                    )
                e = epool.tile([128, 1024], BF16, tag="e", bufs=4, name="e")
                nc.scalar.activation(e, ps, AFT.Exp, scale=SM_SCALE)
                return e

            def emit_av(h, pav, e, g):
                for j in range(2):
                    kt = 2 * g + j
                    nc.tensor.matmul(
                        pav,
                        vT_sb[:, kt, h * 128 : (h + 1) * 128],
                        e[:, j * 512 : (j + 1) * 512],
                        start=(kt == 0),
                        stop=(kt == LT - 1),
                    )

            def finish_unit(st):
                # den broadcast + 1/den + normalize for a completed unit;
                # deferred into the NEXT unit so the PE never waits on the
                # DVE tree.
                h, qc, pav, f = st
                pden = psum.tile([128, 512], F32, tag="pp", bufs=2, name="pden")
                nc.tensor.matmul(pden, ones_bf, f, start=True, stop=True)
                rden = cpool.tile([128, 512], F32, tag="rden", name="rden")
                nc.vector.reciprocal_approx_fast(rden, pden)
                nc.vector.tensor_mul(
                    attn_sb[:, h, qc * 512 : (qc + 1) * 512], pav, rden
                )

            def emit_unit(h, qc, hook=None):
                es = []
                ts = []
                pav = psum.tile([128, 512], F32, tag="av", bufs=2, name="pav")
                es.append(emit_qk(h, qc, 0))
                u0 = None
                for g in range(1, NG + 1):
                    if g < NG:
                        es.append(emit_qk(h, qc, g))
                    if hook is not None:
                        hook(g)
                    emit_av(h, pav, es[g - 1], g - 1)
                    if g % 2 == 0:
                        t = tpool.tile([128, 1024], BF16, tag="t", bufs=6, name="t")
                        nc.vector.tensor_add(t, es[g - 2], es[g - 1])
                        ts.append(t)
                        if g == 4:
                            u0 = tpool.tile(
                                [128, 1024], BF16, tag="t", bufs=6, name="u0"
                            )
                            nc.vector.tensor_add(u0, ts[0], ts[1])
                u1 = tpool.tile([128, 1024], BF16, tag="t", bufs=6, name="u1")
                nc.vector.tensor_add(u1, ts[2], ts[3])
                s = tpool.tile([128, 1024], BF16, tag="t", bufs=6, name="s")
                nc.vector.tensor_add(s, u0, u1)
                f = tpool.tile([128, 512], BF16, tag="f", bufs=2, name="f")
                nc.vector.tensor_add(f, s[:, 0:512], s[:, 512:1024])
                return (h, qc, pav, f)

            pending = None  # completed unit awaiting den/normalize
            deferred_out = None  # qc whose out-projection awaits emission
            for qc in range(LC):
                for h in range(NH):
                    dq = deferred_out if h == 0 and qc > 0 else None
                    if dq is not None:
                        deferred_out = None
                    vjit = qc == 0 and h == 0
                    projh = h + 1 if (qc == 0 and 2 <= h + 1 <= 3) else None
                    pend = pending

                    def hook(g, _dq=dq, _vjit=vjit, _pend=pend, _projh=projh):
                        if _vjit:
                            emit_v_tiles(2 * (g - 1), 2)
                            if g in (2, 4, 6):
                                emit_q0_group(g // 2)
                        elif _projh is not None:
                            emit_proj_group(_projh, g - 1)
                        if g == 2 and _pend is not None:
                            finish_unit(_pend)
                        if _dq is not None and 3 <= g <= 6:
                            emit_out_proj_ot(_dq, g - 3)

                    pending = emit_unit(h, qc, hook)
                    if qc == 0 and h == 0:
                        # head 1's projection (unit 0,0's hooks carry v)
                        for i in range(8):
                            emit_proj_group(1, i)
                deferred_out = qc
            finish_unit(pending)
            for ot in range(CT):
                emit_out_proj_ot(deferred_out, ot)
    nc.compile()
    return nc


_NC_CACHE = {}


def _get_nc():
    if "nc" not in _NC_CACHE:
        nc = bacc.Bacc("TRN2", debug=False)
        build_attn_block(nc)
        _NC_CACHE["nc"] = nc
    return _NC_CACHE["nc"]


def run(trace=False, **inputs):
    nc = _get_nc()
    xs = np.ascontiguousarray(np.asarray(inputs["x"], dtype=np.float32))
    shared = {}
    for nm in ("gn_scale", "gn_bias", "wq", "bq", "wk", "bk", "wv", "bv", "wo", "bo"):
        shared[nm] = np.ascontiguousarray(np.asarray(inputs[nm], dtype=np.float32))
    in_maps = [dict(shared, x=xs[b]) for b in range(B)]
    res = run_bass_kernel_spmd(nc, in_maps, core_ids=list(range(B)), trace=trace)
    out = np.stack([res.results[b]["out"] for b in range(B)], axis=0)
    return out, res


def kernel(**inputs):
    out, _ = run(trace=bool(os.environ.get("ATTN_TRACE")), **inputs)
    return out


# revision 14
# speedup vs baseline: 1.2617x; 1.0011x over previous
"""AttnBlock (GroupNorm + 4-head d=128 self-attention + residual).

Full input x: [8, 512, 2048] fp32. Data-parallel over batch: core b computes
batch b entirely on-chip (no collectives).

Per-core math (C=512, L=2048, G=4 groups, NH=4 heads, HD=128):
  h  = groupnorm(x)          bf16; x stays resident fp32 (residual reuse)
  q  = wq @ h + bq           bf16 [d, l] head-major (PE-transposed weights)
  k  = wk @ h + bk           bf16 [d, l]
  vT = h^T @ wv^T + bv       bf16 [l, d]
  sT[k,q] = k_tile^T q       fp32 PSUM, two k-tiles per 2-bank group
  e = exp(s * scale)         one ACT instr per [128,1024] group -> bf16
  den: DVE bf16 pairwise-add tree over the 8 e-groups -> [128,512],
       one ones-matmul broadcasts the cross-partition sum
  rden = reciprocal_approx_fast(den)   (custom DVE op, ~51 ULP)
  attn = pav * rden          bf16
  out = wo @ attn + bo + x   (DVE STT fuses bias + residual)

All matmul operands bf16 (fp32 PSUM accumulation). Steady state is paced by
the ACT exp stream (~1.15us per [128,1024] group); PE rides just under it,
DVE (den tree + drains) well under.

Scheduling:
 - DMA order: wk0/wq0 row-blocks, then all of x, then wv, wk/wq rest, wo.
 - Groupnorm stats + apply run per channel-tile as x lands; k[0] and
   q[0,qc0] projections accumulate ct-by-ct in the (still unused)
   sA/sB/av PSUM banks so attention starts ~1us after h completes.
 - v projection is emitted just-in-time inside unit (0,0); head h+1's
   k/q projection rides unit (h,0)'s hooks; proj drains go on ACT
   (which idles during the PE-bound qc=0 region).
 - den/rden/normalize of unit U are deferred into unit U+1 (PE never
   waits on the DVE tree); out-projection of qc rides qc+1's first unit.

PSUM budget (8 banks): sA 2 + sB 2 + av 2 + pp 2.
"""

import os
import numpy as np

import concourse.bass as bass
import concourse.tile as tile
from concourse import bacc, mybir
from concourse.bass_utils import run_bass_kernel_spmd
from concourse.masks import make_identity

F32 = mybir.dt.float32
BF16 = mybir.dt.bfloat16

B, C, L = 8, 512, 2048
G = 4            # groupnorm groups; group size 128 == one partition tile
NH, HD = 4, 128  # heads, head dim
CT = C // 128    # 4 channel tiles
LC = L // 512    # 4 l-chunks of 512
LT = L // 128    # 16 l-tiles of 128
NG = LT // 2     # 8 score groups of 2 k-tiles
EPS = 1e-6
SM_SCALE = float(HD) ** -0.5

AFT = mybir.ActivationFunctionType
ALU = mybir.AluOpType


def build_attn_block(nc):
    x_d = nc.dram_tensor("x", [C, L], F32, kind="ExternalInput").ap()
    gs_d = nc.dram_tensor("gn_scale", [C], F32, kind="ExternalInput").ap()
    gb_d = nc.dram_tensor("gn_bias", [C], F32, kind="ExternalInput").ap()
    w_d = {}
    b_d = {}
    for nm in ("q", "k", "v", "o"):
        w_d[nm] = nc.dram_tensor(f"w{nm}", [C, C], F32, kind="ExternalInput").ap()
        b_d[nm] = nc.dram_tensor(f"b{nm}", [C], F32, kind="ExternalInput").ap()
    out_d = nc.dram_tensor("out", [C, L], F32, kind="ExternalOutput").ap()

    with tile.TileContext(nc) as tc:
        with (
            tc.tile_pool(name="const", bufs=1) as const,
            tc.tile_pool(name="wstage", bufs=2) as wstage,
            tc.tile_pool(name="wt", bufs=1) as wt,
            tc.tile_pool(name="big", bufs=1) as big,
            tc.tile_pool(name="small", bufs=4) as small,
            tc.tile_pool(name="epool", bufs=4) as epool,
            tc.tile_pool(name="tpool", bufs=6) as tpool,
            tc.tile_pool(name="cpool", bufs=2) as cpool,
            tc.tile_pool(name="psum", bufs=1, space="PSUM") as psum,
        ):
            # ---- constants ----
            identity = const.tile([128, 128], F32)
            make_identity(nc, identity)
            ones = const.tile([128, 128], F32)
            nc.vector.memset(ones, 1.0)
            ones_bf = const.tile([128, 128], BF16)
            nc.vector.tensor_copy(ones_bf, ones)
            eps_t = const.tile([128, 1], F32)
            nc.vector.memset(eps_t, EPS)

            def load_cvec(name, ap_1d):
                t = const.tile([128, CT], F32, name=name)
                nc.sync.dma_start(out=t, in_=ap_1d.rearrange("(t p) -> p t", p=128))
                return t

            bq_sb = load_cvec("bq_sb", b_d["q"])
            bk_sb = load_cvec("bk_sb", b_d["k"])
            bo_sb = load_cvec("bo_sb", b_d["o"])
            gs_sb = load_cvec("gs_sb", gs_d)
            gb_sb = load_cvec("gb_sb", gb_d)

            bv_bc = const.tile([128, C], F32)  # bv broadcast across partitions
            nc.sync.dma_start(
                out=bv_bc,
                in_=bass.AP(
                    tensor=b_d["v"].tensor,
                    offset=b_d["v"].offset,
                    ap=[[0, 128]] + list(b_d["v"].ap),
                ),
            )

            # ---- big persistent tiles ----
            x_sb = big.tile([128, CT, L], F32, tag="x_sb")
            h_sb = big.tile([128, CT, L], BF16, tag="h_sb")
            q_sb = big.tile([128, NH, L], BF16, tag="q_sb")
            k_sb = big.tile([128, NH, L], BF16, tag="k_sb")
            vT_sb = big.tile([128, LT, C], BF16, tag="vT_sb")
            attn_sb = big.tile([128, NH, L], BF16, tag="attn_sb")

            # ---- weights: DMA row-blocks, PE-transpose into wT[c_in, c_out]
            #      (bf16), drained as one [128, 4, 128] strided DVE copy.
            wts = {}
            for nm in ("q", "k", "v", "o"):
                wts[nm] = wt.tile([128, CT, C], BF16, name=f"w{nm}t")
            wblocks = (
                [("k", 0), ("q", 0)]
                + [("v", ot) for ot in range(CT)]
                + [("k", 1), ("q", 1), ("k", 2), ("q", 2), ("k", 3), ("q", 3)]
                + [("o", ot) for ot in range(CT)]
            )
            wbi = [0]

            def emit_weight_blocks(n):
                for _ in range(n):
                    if wbi[0] >= len(wblocks):
                        return
                    nm, ot = wblocks[wbi[0]]
                    wbi[0] += 1
                    stg = wstage.tile([128, C], F32, tag="stg")
                    nc.sync.dma_start(
                        out=stg, in_=w_d[nm][ot * 128 : (ot + 1) * 128, :]
                    )
                    pt = psum.tile([128, 512], F32, tag="pp", bufs=2, name="pt")
                    for ct in range(CT):
                        nc.tensor.transpose(
                            pt[:, ct * 128 : (ct + 1) * 128],
                            stg[:, ct * 128 : (ct + 1) * 128],
                            identity,
                        )
                    dstw = wts[nm][:, :, ot * 128 : (ot + 1) * 128]
                    nc.vector.tensor_copy(
                        dstw, pt.rearrange("p (c t) -> p c t", c=CT)
                    )

            x_r = x_d.rearrange("(t p) l -> p t l", p=128)

            # head-0 lead-in weights first, then x, then the rest
            emit_weight_blocks(2)
            for ct in range(CT):
                for i2 in range(2):
                    nc.sync.dma_start(
                        out=x_sb[:, ct, i2 * 1024 : (i2 + 1) * 1024],
                        in_=x_r[:, ct, i2 * 1024 : (i2 + 1) * 1024],
                    )

            # k[0] (4 lc) accumulates across the ct loop in the sA/sB banks;
            # q[0, lc0] in an av-tag bank. Attention has not started, so
            # those banks are free; pp stays free for stats pg + transposes.
            ps_k0 = [
                psum.tile([128, 1024], F32, tag="sA", name="ps_k0a"),
                psum.tile([128, 1024], F32, tag="sB", name="ps_k0b"),
            ]
            ps_q0 = psum.tile([128, 512], F32, tag="av", bufs=2, name="ps_q0")

            # ---- groupnorm stats + apply, per channel tile as x lands ----
            for ct in range(CT):
                stats = small.tile([128, 4, 6], F32, tag="stats")
                for i in range(4):
                    nc.vector.bn_stats(
                        out=stats[:, i, :], in_=x_sb[:, ct, i * 512 : (i + 1) * 512]
                    )
                mv = small.tile([128, 2], F32, tag="mv")
                nc.vector.bn_aggr(out=mv, in_=stats)
                stat2 = small.tile([128, 2], F32, tag="stat2")
                nc.vector.tensor_copy(stat2[:, 0:1], mv[:, 0:1])
                nc.vector.scalar_tensor_tensor(
                    out=stat2[:, 1:2],
                    in0=mv[:, 0:1],
                    scalar=mv[:, 0:1],
                    in1=mv[:, 1:2],
                    op0=ALU.mult,
                    op1=ALU.add,
                )
                pg = psum.tile([128, 2], F32, tag="pp", bufs=2, name="pg")
                nc.tensor.matmul(pg, ones, stat2, start=True, stop=True)
                mean_t = small.tile([128, 1], F32, tag="mean_t")
                nc.vector.tensor_scalar_mul(mean_t, pg[:, 0:1], 1.0 / 128.0)
                ex2_t = small.tile([128, 1], F32, tag="ex2_t")
                nc.vector.tensor_scalar_mul(ex2_t, pg[:, 1:2], 1.0 / 128.0)
                var_t = small.tile([128, 1], F32, tag="var_t")
                nc.vector.tensor_mul(var_t, mean_t, mean_t)
                nc.vector.tensor_sub(var_t, ex2_t, var_t)
                # rstd = exp(-0.5*ln(var+eps)): keeps the whole kernel in the
                # natural_log_exp ACT table set (no mid-kernel table swap)
                lnv_t = small.tile([128, 1], F32, tag="lnv_t")
                nc.scalar.activation(lnv_t, var_t, AFT.Ln, bias=eps_t)
                rstd_t = small.tile([128, 1], F32, tag="rstd_t")
                nc.scalar.activation(rstd_t, lnv_t, AFT.Exp, scale=-0.5)
                a_t = small.tile([128, 1], F32, tag="a_t", bufs=CT)
                nc.vector.tensor_mul(a_t, rstd_t, gs_sb[:, ct : ct + 1])
                b_t = small.tile([128, 1], F32, tag="b_t", bufs=CT)
                nc.vector.tensor_mul(b_t, mean_t, a_t)
                nc.vector.tensor_sub(b_t, gb_sb[:, ct : ct + 1], b_t)
                # h for this channel tile (ACT, one big instr)
                nc.scalar.activation(
                    h_sb[:, ct, :], x_sb[:, ct, :], AFT.Identity, bias=b_t, scale=a_t
                )
                # partial k[0] / q[0,lc0] accumulation on this channel tile
                for lc in range(LC):
                    nc.tensor.matmul(
                        ps_k0[lc // 2][:, (lc % 2) * 512 : (lc % 2 + 1) * 512],
                        wts["k"][:, ct, 0:128],
                        h_sb[:, ct, lc * 512 : (lc + 1) * 512],
                        start=(ct == 0),
                        stop=(ct == CT - 1),
                    )
                nc.tensor.matmul(
                    ps_q0,
                    wts["q"][:, ct, 0:128],
                    h_sb[:, ct, 0:512],
                    start=(ct == 0),
                    stop=(ct == CT - 1),
                )

            # drain k[0] / q[0,lc0]: split ACT/DVE so neither serializes
            for lc in range(2):
                nc.scalar.activation(
                    k_sb[:, 0, lc * 512 : (lc + 1) * 512],
                    ps_k0[0][:, lc * 512 : (lc + 1) * 512],
                    AFT.Identity,
                    bias=bk_sb[:, 0:1],
                )
            for lc in range(2, LC):
                nc.vector.tensor_scalar_add(
                    k_sb[:, 0, lc * 512 : (lc + 1) * 512],
                    ps_k0[1][:, (lc - 2) * 512 : (lc - 1) * 512],
                    bk_sb[:, 0:1],
                )
            nc.vector.tensor_scalar_add(q_sb[:, 0, 0:512], ps_q0, bq_sb[:, 0:1])
            # remaining weights: DMA'd only now, so x had full HBM bandwidth
            emit_weight_blocks(len(wblocks))

            # ---- projection helpers (drains on ACT: it idles during the
            #      PE-bound qc=0 region) ----
            def emit_proj_group(h, i):
                # i in 0..7: 0-3 -> k lc=i, 4-7 -> q lc=i-4
                dst, wtt, bias = (
                    (k_sb, wts["k"], bk_sb) if i < 4 else (q_sb, wts["q"], bq_sb)
                )
                lc = i % 4
                pp = psum.tile([128, 512], F32, tag="pp", bufs=2, name="pp")
                for ct in range(CT):
                    nc.tensor.matmul(
                        pp,
                        wtt[:, ct, h * 128 : (h + 1) * 128],
                        h_sb[:, ct, lc * 512 : (lc + 1) * 512],
                        start=(ct == 0),
                        stop=(ct == CT - 1),
                    )
                nc.vector.tensor_scalar_add(
                    dst[:, h, lc * 512 : (lc + 1) * 512], pp, bias[:, h : h + 1]
                )

            def emit_q0_group(lc):
                pp = psum.tile([128, 512], F32, tag="pp", bufs=2, name="pp")
                for ct in range(CT):
                    nc.tensor.matmul(
                        pp,
                        wts["q"][:, ct, 0:128],
                        h_sb[:, ct, lc * 512 : (lc + 1) * 512],
                        start=(ct == 0),
                        stop=(ct == CT - 1),
                    )
                nc.vector.tensor_scalar_add(
                    q_sb[:, 0, lc * 512 : (lc + 1) * 512], pp, bq_sb[:, 0:1]
                )

            def emit_v_tiles(lt0, n):
                for lt in range(lt0, lt0 + n):
                    pv = psum.tile([128, 512], F32, tag="pp", bufs=2, name="pv")
                    for ct in range(CT):
                        nc.tensor.matmul(
                            pv,
                            h_sb[:, ct, lt * 128 : (lt + 1) * 128],
                            wts["v"][:, ct, :],
                            start=(ct == 0),
                            stop=(ct == CT - 1),
                        )
                    nc.vector.tensor_add(vT_sb[:, lt, :], pv, bv_bc)

            def emit_out_proj_ot(qc, ot):
                pop = psum.tile([128, 512], F32, tag="pp", bufs=2, name="pop")
                for ct in range(CT):
                    nc.tensor.matmul(
                        pop,
                        wts["o"][:, ct, ot * 128 : (ot + 1) * 128],
                        attn_sb[:, ct, qc * 512 : (qc + 1) * 512],
                        start=(ct == 0),
                        stop=(ct == CT - 1),
                    )
                ot_sb = cpool.tile([128, 512], F32, tag="ot_sb")
                nc.vector.scalar_tensor_tensor(
                    out=ot_sb,
                    in0=pop,
                    scalar=bo_sb[:, ot : ot + 1],
                    in1=x_sb[:, ot, qc * 512 : (qc + 1) * 512],
                    op0=ALU.add,
                    op1=ALU.add,
                )
                nc.sync.dma_start(
                    out=out_d[ot * 128 : (ot + 1) * 128, qc * 512 : (qc + 1) * 512],
                    in_=ot_sb,
                )

            # ---- attention ----
            def emit_qk(h, qc, g):
                ps = psum.tile(
                    [128, 1024], F32, tag=("sA" if g % 2 == 0 else "sB"), name="ps"
                )
                for j in range(2):
                    kt = 2 * g + j
                    nc.tensor.matmul(
                        ps[:, j * 512 : (j + 1) * 512],
                        k_sb[:, h, kt * 128 : (kt + 1) * 128],
                        q_sb[:, h, qc * 512 : (qc + 1) * 512],
                        start=True,
                        stop=True,
                    )
                e = epool.tile([128, 1024], BF16, tag="e", bufs=4, name="e")
                nc.scalar.activation(e, ps, AFT.Exp, scale=SM_SCALE)
                return e

            def emit_av(h, pav, e, g):
                for j in range(2):
                    kt = 2 * g + j
                    nc.tensor.matmul(
                        pav,
                        vT_sb[:, kt, h * 128 : (h + 1) * 128],
                        e[:, j * 512 : (j + 1) * 512],
                        start=(kt == 0),
                        stop=(kt == LT - 1),
                    )

            def finish_unit(st):
                # den broadcast + 1/den + normalize for a completed unit;
                # deferred into the NEXT unit so the PE never waits on the
                # DVE tree.
                h, qc, pav, f = st
                pden = psum.tile([128, 512], F32, tag="pp", bufs=2, name="pden")
                nc.tensor.matmul(pden, ones_bf, f, start=True, stop=True)
                rden = cpool.tile([128, 512], F32, tag="rden", name="rden")
                nc.vector.reciprocal_approx_fast(rden, pden)
                nc.vector.tensor_mul(
                    attn_sb[:, h, qc * 512 : (qc + 1) * 512], pav, rden
                )

            def emit_unit(h, qc, hook=None, pe_den=False):
                es = []
                ts = []
                pav = psum.tile([128, 512], F32, tag="av", bufs=2, name="pav")
                pden = None
                if pe_den:
                    # last unit: accumulate den on the PE as e-groups land, so
                    # the finish isn't gated on the serial DVE tree at the tail
                    pden = psum.tile([128, 512], F32, tag="pp", bufs=2, name="pden")
                es.append(emit_qk(h, qc, 0))
                u0 = None
                for g in range(1, NG + 1):
                    if g < NG:
                        es.append(emit_qk(h, qc, g))
                    if hook is not None:
                        hook(g)
                    emit_av(h, pav, es[g - 1], g - 1)
                    if pe_den:
                        for j in range(2):
                            kt = 2 * (g - 1) + j
                            nc.tensor.matmul(
                                pden,
                                ones_bf,
                                es[g - 1][:, j * 512 : (j + 1) * 512],
                                start=(kt == 0),
                                stop=(kt == LT - 1),
                            )
                        continue
                    if g % 2 == 0:
                        t = tpool.tile([128, 1024], BF16, tag="t", bufs=6, name="t")
                        nc.vector.tensor_add(t, es[g - 2], es[g - 1])
                        ts.append(t)
                        if g == 4:
                            u0 = tpool.tile(
                                [128, 1024], BF16, tag="t", bufs=6, name="u0"
                            )
                            nc.vector.tensor_add(u0, ts[0], ts[1])
                if pe_den:
                    rden = cpool.tile([128, 512], F32, tag="rden", name="rden")
                    nc.vector.reciprocal_approx_fast(rden, pden)
                    nc.vector.tensor_mul(
                        attn_sb[:, h, qc * 512 : (qc + 1) * 512], pav, rden
                    )
                    return None
                u1 = tpool.tile([128, 1024], BF16, tag="t", bufs=6, name="u1")
                nc.vector.tensor_add(u1, ts[2], ts[3])
                s = tpool.tile([128, 1024], BF16, tag="t", bufs=6, name="s")
                nc.vector.tensor_add(s, u0, u1)
                f = tpool.tile([128, 512], BF16, tag="f", bufs=2, name="f")
                nc.vector.tensor_add(f, s[:, 0:512], s[:, 512:1024])
                return (h, qc, pav, f)

            pending = None  # completed unit awaiting den/normalize
            deferred_out = None  # qc whose out-projection awaits emission
            for qc in range(LC):
                for h in range(NH):
                    dq = deferred_out if h == 0 and qc > 0 else None
                    if dq is not None:
                        deferred_out = None
                    vjit = qc == 0 and h == 0
                    projh = h + 1 if (qc == 0 and 2 <= h + 1 <= 3) else None
                    pend = pending

                    def hook(g, _dq=dq, _vjit=vjit, _pend=pend, _projh=projh):
                        if _vjit:
                            emit_v_tiles(2 * (g - 1), 2)
                            if g in (2, 4, 6):
                                emit_q0_group(g // 2)
                        elif _projh is not None:
                            emit_proj_group(_projh, g - 1)
                        if g == 2 and _pend is not None:
                            finish_unit(_pend)
                        if _dq is not None and 3 <= g <= 6:
                            emit_out_proj_ot(_dq, g - 3)

                    last = qc == LC - 1 and h == NH - 1
                    pending = emit_unit(h, qc, hook, pe_den=last)
                    if qc == 0 and h == 0:
                        # head 1's projection (unit 0,0's hooks carry v)
                        for i in range(8):
                            emit_proj_group(1, i)
                deferred_out = qc
            for ot in range(CT):
                emit_out_proj_ot(deferred_out, ot)
    nc.compile()
    return nc


_NC_CACHE = {}


def _get_nc():
    if "nc" not in _NC_CACHE:
        nc = bacc.Bacc("TRN2", debug=False)
        build_attn_block(nc)
        _NC_CACHE["nc"] = nc
    return _NC_CACHE["nc"]


def run(trace=False, **inputs):
    nc = _get_nc()
    xs = np.ascontiguousarray(np.asarray(inputs["x"], dtype=np.float32))
    shared = {}
    for nm in ("gn_scale", "gn_bias", "wq", "bq", "wk", "bk", "wv", "bv", "wo", "bo"):
        shared[nm] = np.ascontiguousarray(np.asarray(inputs[nm], dtype=np.float32))
    in_maps = [dict(shared, x=xs[b]) for b in range(B)]
    res = run_bass_kernel_spmd(nc, in_maps, core_ids=list(range(B)), trace=trace)
    out = np.stack([res.results[b]["out"] for b in range(B)], axis=0)
    return out, res


def kernel(**inputs):
    out, _ = run(trace=bool(os.environ.get("ATTN_TRACE")), **inputs)
    return out


# revision 19
# speedup vs baseline: 1.3093x; 1.0377x over previous
"""AttnBlock (GroupNorm + 4-head d=128 self-attention + residual).

Full input x: [8, 512, 2048] fp32. Data-parallel over batch: core b computes
batch b entirely on-chip (no collectives).

Per-core math (C=512, L=2048, G=4 groups, NH=4 heads, HD=128):
  h  = groupnorm(x)          bf16; x stays resident fp32 (residual reuse)
  q  = wq @ h + bq           bf16 [d, l] head-major (PE-transposed weights)
  k  = wk @ h + bk           bf16 [d, l]
  vT = h^T @ wv^T + bv       bf16 [l, d]
  sT[k,q] = k_tile^T q       fp32 PSUM, two k-tiles per 2-bank group
  e = exp(s * scale)         one ACT instr per [128,1024] group -> bf16
  den: DVE bf16 pairwise-add tree over the 8 e-groups -> [128,512],
       one ones-matmul broadcasts the cross-partition sum
  rden = reciprocal_approx_fast(den)   (custom DVE op, ~51 ULP)
  attn = pav * rden          bf16
  out = wo @ attn + bo + x   (DVE STT fuses bias + residual)

All matmul operands bf16 (fp32 PSUM accumulation). Steady state is paced by
the ACT exp stream (~1.15us per [128,1024] group); PE rides just under it,
DVE (den tree + drains) well under.

Scheduling:
 - DMA order: wk0/wq0 row-blocks, then all of x, then wv, wk/wq rest, wo.
 - Groupnorm stats + apply run per channel-tile as x lands; k[0] and
   q[0,qc0] projections accumulate ct-by-ct in the (still unused)
   sA/sB/av PSUM banks so attention starts ~1us after h completes.
 - v projection is emitted just-in-time inside unit (0,0); head h+1's
   k/q projection rides unit (h,0)'s hooks; proj drains go on ACT
   (which idles during the PE-bound qc=0 region).
 - den/rden/normalize of unit U are deferred into unit U+1 (PE never
   waits on the DVE tree); out-projection of qc rides qc+1's first unit.

PSUM budget (8 banks): sA 2 + sB 2 + av 2 + pp 2.
"""

import os
import numpy as np

import concourse.bass as bass
import concourse.tile as tile
from concourse import bacc, mybir
from concourse.bass_utils import run_bass_kernel_spmd
from concourse.masks import make_identity

F32 = mybir.dt.float32
BF16 = mybir.dt.bfloat16

B, C, L = 8, 512, 2048
G = 4            # groupnorm groups; group size 128 == one partition tile
NH, HD = 4, 128  # heads, head dim
CT = C // 128    # 4 channel tiles
LC = L // 512    # 4 l-chunks of 512
LT = L // 128    # 16 l-tiles of 128
NG = LT // 2     # 8 score groups of 2 k-tiles
EPS = 1e-6
SM_SCALE = float(HD) ** -0.5

AFT = mybir.ActivationFunctionType
ALU = mybir.AluOpType


def build_attn_block(nc):
    x_d = nc.dram_tensor("x", [C, L], F32, kind="ExternalInput").ap()
    gs_d = nc.dram_tensor("gn_scale", [C], F32, kind="ExternalInput").ap()
    gb_d = nc.dram_tensor("gn_bias", [C], F32, kind="ExternalInput").ap()
    w_d = {}
    b_d = {}
    for nm in ("q", "k", "v", "o"):
        w_d[nm] = nc.dram_tensor(f"w{nm}", [C, C], F32, kind="ExternalInput").ap()
        b_d[nm] = nc.dram_tensor(f"b{nm}", [C], F32, kind="ExternalInput").ap()
    out_d = nc.dram_tensor("out", [C, L], F32, kind="ExternalOutput").ap()

    with tile.TileContext(nc) as tc:
        with (
            tc.tile_pool(name="const", bufs=1) as const,
            tc.tile_pool(name="wstage", bufs=2) as wstage,
            tc.tile_pool(name="wt", bufs=1) as wt,
            tc.tile_pool(name="big", bufs=1) as big,
            tc.tile_pool(name="small", bufs=4) as small,
            tc.tile_pool(name="epool", bufs=4) as epool,
            tc.tile_pool(name="tpool", bufs=6) as tpool,
            tc.tile_pool(name="cpool", bufs=2) as cpool,
            tc.tile_pool(name="psum", bufs=1, space="PSUM") as psum,
        ):
            # ---- constants ----
            identity = const.tile([128, 128], F32)
            make_identity(nc, identity)
            ones = const.tile([128, 128], F32)
            nc.vector.memset(ones, 1.0)
            ones_bf = const.tile([128, 128], BF16)
            nc.vector.tensor_copy(ones_bf, ones)
            eps_t = const.tile([128, 1], F32)
            nc.vector.memset(eps_t, EPS)

            def load_cvec(name, ap_1d):
                t = const.tile([128, CT], F32, name=name)
                nc.sync.dma_start(out=t, in_=ap_1d.rearrange("(t p) -> p t", p=128))
                return t

            bq_sb = load_cvec("bq_sb", b_d["q"])
            bk_sb = load_cvec("bk_sb", b_d["k"])
            bo_sb = load_cvec("bo_sb", b_d["o"])
            gs_sb = load_cvec("gs_sb", gs_d)
            gb_sb = load_cvec("gb_sb", gb_d)

            bv_bc = const.tile([128, C], F32)  # bv broadcast across partitions
            nc.sync.dma_start(
                out=bv_bc,
                in_=bass.AP(
                    tensor=b_d["v"].tensor,
                    offset=b_d["v"].offset,
                    ap=[[0, 128]] + list(b_d["v"].ap),
                ),
            )

            # ---- big persistent tiles ----
            x_sb = big.tile([128, CT, L], F32, tag="x_sb")
            h_sb = big.tile([128, CT, L], BF16, tag="h_sb")
            q_sb = big.tile([128, NH, L], BF16, tag="q_sb")
            k_sb = big.tile([128, NH, L], BF16, tag="k_sb")
            vT_sb = big.tile([128, LT, C], BF16, tag="vT_sb")
            attn_sb = big.tile([128, NH, L], BF16, tag="attn_sb")

            # ---- weights: DMA row-blocks, PE-transpose into wT[c_in, c_out]
            #      (bf16), drained as one [128, 4, 128] strided DVE copy.
            wts = {}
            for nm in ("q", "k", "v", "o"):
                wts[nm] = wt.tile([128, CT, C], BF16, name=f"w{nm}t")
            wblocks = (
                [("k", 0), ("q", 0)]
                + [("v", ot) for ot in range(CT)]  # before x: unit-0 v-JIT
                + [("k", 1), ("q", 1), ("k", 2), ("q", 2), ("k", 3), ("q", 3)]
                + [("o", ot) for ot in range(CT)]
            )
            wbi = [0]

            def emit_weight_blocks(n):
                for _ in range(n):
                    if wbi[0] >= len(wblocks):
                        return
                    nm, ot = wblocks[wbi[0]]
                    wbi[0] += 1
                    stg = wstage.tile([128, C], F32, tag="stg")
                    nc.sync.dma_start(
                        out=stg, in_=w_d[nm][ot * 128 : (ot + 1) * 128, :]
                    )
                    pt = psum.tile([128, 512], F32, tag="pp", bufs=2, name="pt")
                    for ct in range(CT):
                        nc.tensor.transpose(
                            pt[:, ct * 128 : (ct + 1) * 128],
                            stg[:, ct * 128 : (ct + 1) * 128],
                            identity,
                        )
                    dstw = wts[nm][:, :, ot * 128 : (ot + 1) * 128]
                    nc.vector.tensor_copy(
                        dstw, pt.rearrange("p (c t) -> p c t", c=CT)
                    )

            x_r = x_d.rearrange("(t p) l -> p t l", p=128)

            # head-0 lead-in + v weights first, then x, then the rest
            emit_weight_blocks(6)
            for ct in range(CT):
                for i2 in range(2):
                    nc.sync.dma_start(
                        out=x_sb[:, ct, i2 * 1024 : (i2 + 1) * 1024],
                        in_=x_r[:, ct, i2 * 1024 : (i2 + 1) * 1024],
                    )

            # k[0] (4 lc) accumulates across the ct loop in the sA/sB banks;
            # q[0, lc0] in an av-tag bank. Attention has not started, so
            # those banks are free; pp stays free for stats pg + transposes.
            ps_k0 = [
                psum.tile([128, 1024], F32, tag="sA", name="ps_k0a"),
                psum.tile([128, 1024], F32, tag="sB", name="ps_k0b"),
            ]
            ps_q0 = psum.tile([128, 512], F32, tag="av", bufs=2, name="ps_q0")

            # ---- groupnorm stats + apply, per channel tile as x lands ----
            for ct in range(CT):
                stats = small.tile([128, 4, 6], F32, tag="stats")
                for i in range(4):
                    nc.vector.bn_stats(
                        out=stats[:, i, :], in_=x_sb[:, ct, i * 512 : (i + 1) * 512]
                    )
                mv = small.tile([128, 2], F32, tag="mv")
                nc.vector.bn_aggr(out=mv, in_=stats)
                stat2 = small.tile([128, 2], F32, tag="stat2")
                nc.vector.tensor_copy(stat2[:, 0:1], mv[:, 0:1])
                nc.vector.scalar_tensor_tensor(
                    out=stat2[:, 1:2],
                    in0=mv[:, 0:1],
                    scalar=mv[:, 0:1],
                    in1=mv[:, 1:2],
                    op0=ALU.mult,
                    op1=ALU.add,
                )
                pg = psum.tile([128, 2], F32, tag="pp", bufs=2, name="pg")
                nc.tensor.matmul(pg, ones, stat2, start=True, stop=True)
                mean_t = small.tile([128, 1], F32, tag="mean_t")
                nc.vector.tensor_scalar_mul(mean_t, pg[:, 0:1], 1.0 / 128.0)
                ex2_t = small.tile([128, 1], F32, tag="ex2_t")
                nc.vector.tensor_scalar_mul(ex2_t, pg[:, 1:2], 1.0 / 128.0)
                var_t = small.tile([128, 1], F32, tag="var_t")
                nc.vector.tensor_mul(var_t, mean_t, mean_t)
                nc.vector.tensor_sub(var_t, ex2_t, var_t)
                std_t = small.tile([128, 1], F32, tag="std_t")
                nc.scalar.activation(std_t, var_t, AFT.Sqrt, bias=eps_t)
                rstd_t = small.tile([128, 1], F32, tag="rstd_t")
                nc.vector.reciprocal(rstd_t, std_t)
                a_t = small.tile([128, 1], F32, tag="a_t", bufs=CT)
                nc.vector.tensor_mul(a_t, rstd_t, gs_sb[:, ct : ct + 1])
                b_t = small.tile([128, 1], F32, tag="b_t", bufs=CT)
                nc.vector.tensor_mul(b_t, mean_t, a_t)
                nc.vector.tensor_sub(b_t, gb_sb[:, ct : ct + 1], b_t)
                # h for this channel tile (ACT, one big instr)
                nc.scalar.activation(
                    h_sb[:, ct, :], x_sb[:, ct, :], AFT.Identity, bias=b_t, scale=a_t
                )
                # partial k[0] / q[0,lc0] accumulation on this channel tile
                for lc in range(LC):
                    nc.tensor.matmul(
                        ps_k0[lc // 2][:, (lc % 2) * 512 : (lc % 2 + 1) * 512],
                        wts["k"][:, ct, 0:128],
                        h_sb[:, ct, lc * 512 : (lc + 1) * 512],
                        start=(ct == 0),
                        stop=(ct == CT - 1),
                    )
                nc.tensor.matmul(
                    ps_q0,
                    wts["q"][:, ct, 0:128],
                    h_sb[:, ct, 0:512],
                    start=(ct == 0),
                    stop=(ct == CT - 1),
                )

            # preload the exp table set (all Sqrt uses are behind us in the
            # ACT stream, so the set is loaded exactly twice)
            dummy = small.tile([128, 1], F32, tag="dummy")
            nc.scalar.activation(dummy, eps_t, AFT.Exp)

            # drain k[0] / q[0,lc0]: split ACT/DVE so neither serializes
            for lc in range(2):
                nc.scalar.activation(
                    k_sb[:, 0, lc * 512 : (lc + 1) * 512],
                    ps_k0[0][:, lc * 512 : (lc + 1) * 512],
                    AFT.Identity,
                    bias=bk_sb[:, 0:1],
                )
            for lc in range(2, LC):
                nc.vector.tensor_scalar_add(
                    k_sb[:, 0, lc * 512 : (lc + 1) * 512],
                    ps_k0[1][:, (lc - 2) * 512 : (lc - 1) * 512],
                    bk_sb[:, 0:1],
                )
            nc.vector.tensor_scalar_add(q_sb[:, 0, 0:512], ps_q0, bq_sb[:, 0:1])
            # remaining weights: DMA'd only now, so x had full HBM bandwidth
            emit_weight_blocks(len(wblocks))

            # ---- projection helpers (drains on ACT: it idles during the
            #      PE-bound qc=0 region) ----
            def emit_proj_group(h, i):
                # i in 0..7: 0-3 -> k lc=i, 4-7 -> q lc=i-4
                dst, wtt, bias = (
                    (k_sb, wts["k"], bk_sb) if i < 4 else (q_sb, wts["q"], bq_sb)
                )
                lc = i % 4
                pp = psum.tile([128, 512], F32, tag="pp", bufs=2, name="pp")
                for ct in range(CT):
                    nc.tensor.matmul(
                        pp,
                        wtt[:, ct, h * 128 : (h + 1) * 128],
                        h_sb[:, ct, lc * 512 : (lc + 1) * 512],
                        start=(ct == 0),
                        stop=(ct == CT - 1),
                    )
                nc.vector.tensor_scalar_add(
                    dst[:, h, lc * 512 : (lc + 1) * 512], pp, bias[:, h : h + 1]
                )

            def emit_q0_group(lc):
                pp = psum.tile([128, 512], F32, tag="pp", bufs=2, name="pp")
                for ct in range(CT):
                    nc.tensor.matmul(
                        pp,
                        wts["q"][:, ct, 0:128],
                        h_sb[:, ct, lc * 512 : (lc + 1) * 512],
                        start=(ct == 0),
                        stop=(ct == CT - 1),
                    )
                nc.vector.tensor_scalar_add(
                    q_sb[:, 0, lc * 512 : (lc + 1) * 512], pp, bq_sb[:, 0:1]
                )

            def emit_v_tiles(lt0, n):
                for lt in range(lt0, lt0 + n):
                    pv = psum.tile([128, 512], F32, tag="pp", bufs=2, name="pv")
                    for ct in range(CT):
                        nc.tensor.matmul(
                            pv,
                            h_sb[:, ct, lt * 128 : (lt + 1) * 128],
                            wts["v"][:, ct, :],
                            start=(ct == 0),
                            stop=(ct == CT - 1),
                        )
                    nc.vector.tensor_add(vT_sb[:, lt, :], pv, bv_bc)

            def emit_out_proj_ot(qc, ot):
                pop = psum.tile([128, 512], F32, tag="pp", bufs=2, name="pop")
                for ct in range(CT):
                    nc.tensor.matmul(
                        pop,
                        wts["o"][:, ct, ot * 128 : (ot + 1) * 128],
                        attn_sb[:, ct, qc * 512 : (qc + 1) * 512],
                        start=(ct == 0),
                        stop=(ct == CT - 1),
                    )
                ot_sb = cpool.tile([128, 512], F32, tag="ot_sb")
                nc.vector.scalar_tensor_tensor(
                    out=ot_sb,
                    in0=pop,
                    scalar=bo_sb[:, ot : ot + 1],
                    in1=x_sb[:, ot, qc * 512 : (qc + 1) * 512],
                    op0=ALU.add,
                    op1=ALU.add,
                )
                nc.sync.dma_start(
                    out=out_d[ot * 128 : (ot + 1) * 128, qc * 512 : (qc + 1) * 512],
                    in_=ot_sb,
                )

            # ---- attention ----
            def emit_qk(h, qc, g):
                ps = psum.tile(
                    [128, 1024], F32, tag=("sA" if g % 2 == 0 else "sB"), name="ps"
                )
                for j in range(2):
                    kt = 2 * g + j
                    nc.tensor.matmul(
                        ps[:, j * 512 : (j + 1) * 512],
                        k_sb[:, h, kt * 128 : (kt + 1) * 128],
                        q_sb[:, h, qc * 512 : (qc + 1) * 512],
                        start=True,
                        stop=True,
                    )
                e = epool.tile([128, 1024], BF16, tag="e", bufs=4, name="e")
                nc.scalar.activation(e, ps, AFT.Exp, scale=SM_SCALE)
                return e

            def emit_av(h, pav, e, g):
                for j in range(2):
                    kt = 2 * g + j
                    nc.tensor.matmul(
                        pav,
                        vT_sb[:, kt, h * 128 : (h + 1) * 128],
                        e[:, j * 512 : (j + 1) * 512],
                        start=(kt == 0),
                        stop=(kt == LT - 1),
                    )

            def finish_unit(st):
                # den broadcast + 1/den + normalize for a completed unit;
                # deferred into the NEXT unit so the PE never waits on the
                # DVE tree.
                h, qc, pav, f = st
                pden = psum.tile([128, 512], F32, tag="pp", bufs=2, name="pden")
                nc.tensor.matmul(pden, ones_bf, f, start=True, stop=True)
                rden = cpool.tile([128, 512], F32, tag="rden", name="rden")
                nc.vector.reciprocal_approx_fast(rden, pden)
                nc.vector.tensor_mul(
                    attn_sb[:, h, qc * 512 : (qc + 1) * 512], pav, rden
                )

            def emit_unit(h, qc, hook=None, pe_den=False):
                es = []
                ts = []
                pav = psum.tile([128, 512], F32, tag="av", bufs=2, name="pav")
                pden = None
                if pe_den:
                    # last unit: accumulate den on the PE as e-groups land, so
                    # the finish isn't gated on the serial DVE tree at the tail
                    pden = psum.tile([128, 512], F32, tag="pp", bufs=2, name="pden")
                es.append(emit_qk(h, qc, 0))
                u0 = None
                for g in range(1, NG + 1):
                    if g < NG:
                        es.append(emit_qk(h, qc, g))
                    if hook is not None:
                        hook(g)
                    emit_av(h, pav, es[g - 1], g - 1)
                    if pe_den:
                        for j in range(2):
                            kt = 2 * (g - 1) + j
                            nc.tensor.matmul(
                                pden,
                                ones_bf,
                                es[g - 1][:, j * 512 : (j + 1) * 512],
                                start=(kt == 0),
                                stop=(kt == LT - 1),
                            )
                        continue
                    if g % 2 == 0:
                        t = tpool.tile([128, 1024], BF16, tag="t", bufs=6, name="t")
                        nc.vector.tensor_add(t, es[g - 2], es[g - 1])
                        ts.append(t)
                        if g == 4:
                            u0 = tpool.tile(
                                [128, 1024], BF16, tag="t", bufs=6, name="u0"
                            )
                            nc.vector.tensor_add(u0, ts[0], ts[1])
                if pe_den:
                    rden = cpool.tile([128, 512], F32, tag="rden", name="rden")
                    nc.vector.reciprocal_approx_fast(rden, pden)
                    nc.vector.tensor_mul(
                        attn_sb[:, h, qc * 512 : (qc + 1) * 512], pav, rden
                    )
                    return None
                u1 = tpool.tile([128, 1024], BF16, tag="t", bufs=6, name="u1")
                nc.vector.tensor_add(u1, ts[2], ts[3])
                s = tpool.tile([128, 1024], BF16, tag="t", bufs=6, name="s")
                nc.vector.tensor_add(s, u0, u1)
                f = tpool.tile([128, 512], BF16, tag="f", bufs=2, name="f")
                nc.vector.tensor_add(f, s[:, 0:512], s[:, 512:1024])
                return (h, qc, pav, f)

            pending = None  # completed unit awaiting den/normalize
            deferred_out = None  # qc whose out-projection awaits emission
            for qc in range(LC):
                for h in range(NH):
                    dq = deferred_out if h <= 1 and qc > 0 else None
                    if h == 1:
                        deferred_out = None
                    vjit = qc == 0 and h == 0
                    projh = h + 1 if (qc == 0 and 2 <= h + 1 <= 3) else None
                    pend = pending

                    def hook(g, _dq=dq, _vjit=vjit, _pend=pend, _projh=projh, _h=h):
                        if _vjit:
                            emit_v_tiles(2 * (g - 1), 2)
                            if g in (2, 4, 6):
                                emit_q0_group(g // 2)
                        elif _projh is not None:
                            emit_proj_group(_projh, g - 1)
                        if g == 2 and _pend is not None:
                            finish_unit(_pend)
                        if _dq is not None and g in (4, 6):
                            emit_out_proj_ot(_dq, _h * 2 + g // 2 - 2)

                    last = qc == LC - 1 and h == NH - 1
                    pending = emit_unit(h, qc, hook, pe_den=last)
                    if qc == 0 and h == 0:
                        # head 1's projection (unit 0,0's hooks carry v)
                        for i in range(8):
                            emit_proj_group(1, i)
                deferred_out = qc
            for ot in range(CT):
                emit_out_proj_ot(deferred_out, ot)
    nc.compile()
    return nc


_NC_CACHE = {}


def _get_nc():
    if "nc" not in _NC_CACHE:
        nc = bacc.Bacc("TRN2", debug=False)
        build_attn_block(nc)
        _NC_CACHE["nc"] = nc
    return _NC_CACHE["nc"]


def run(trace=False, **inputs):
    nc = _get_nc()
    xs = np.ascontiguousarray(np.asarray(inputs["x"], dtype=np.float32))
    shared = {}
    for nm in ("gn_scale", "gn_bias", "wq", "bq", "wk", "bk", "wv", "bv", "wo", "bo"):
        shared[nm] = np.ascontiguousarray(np.asarray(inputs[nm], dtype=np.float32))
    in_maps = [dict(shared, x=xs[b]) for b in range(B)]
    res = run_bass_kernel_spmd(nc, in_maps, core_ids=list(range(B)), trace=trace)
    out = np.stack([res.results[b]["out"] for b in range(B)], axis=0)
    return out, res


def kernel(**inputs):
    out, _ = run(trace=bool(os.environ.get("ATTN_TRACE")), **inputs)
    return out
